# revision 3
# baseline (speedup 1.0000x reference)
"""Local (Gaussian-banded) attention kernel for Trainium2, 8 NeuronCores. v2

Math: out = rownorm(gauss_band(sigma)) @ (x @ Wg) @ Wout
The Gaussian positional mask with sigma in [0.5, 2.5] decays below fp32
resolution past |i-j| > 32, so attention is a 65-tap banded matmul.

Sharding: core c = (batch b = c//2, seq-half s = c%2). Each core computes
out rows [s*1024, (s+1)*1024) of its batch. s=1 halves are row-reversed on
host so the sequence edge is always at local row 0 -> all 8 cores run the
same program with the same band constants (pure SPMD).

Performance design (measured on HW, ~59us vs 89us fp32r baseline):
  - all matmul operands bf16: full-rate PE at any stream width, ~95ns
    weight loads, half the DMA bytes (end-to-end rel err 4.1e-3 vs the
    2e-2 gate)
  - host-pre-arranged DRAM layouts ([128, X] SBUF-partition order) so
    input DMA uses few, large descriptors; the early DMA phase is
    descriptor-rate-limited, not byte-limited
  - input DMAs chunked + ordered so the first stage-1 matmul needs only
    ~0.6 MB; dependency granularity is per dma_start
  - PE warmup matmuls on a memset tile bridge the DMA prologue: the DVFS
    ramp (~3us of continuous busy to reach 2.4 GHz, reset by any idle
    gap) completes before stage 1 instead of during it
  - stage-2 band streams trimmed to their true column support
    (A: cols 0:128, B: 64:256, C: 192:256 of each 256-row chunk)
  - sweep 0 of stage 2 interleaved into the stage-1 tile loop (its
    weight loads hide under stage-1 streams); sweeps 1-3 tile-major with
    stage-3 row-pairs inlined one tile after their chunk closes
  - PSUM->SBUF evacuation split across vector + scalar engines; scalar
    activation table pre-warmed during the prologue
  - xT zero-padded to 1152 rows so tile 8 is a full 128-row tile
  - output DMA per 128-row chunk for early drain
"""

import sys

for _p in ("/opt/trn_rl_repo", "/root/.axon_site/_ro/trn_rl_repo"):
    if _p not in sys.path:
        sys.path.append(_p)

import numpy as np
import ml_dtypes

B, N, D = 4, 2048, 512
H, DH = 8, 128
INNER = H * DH
W = 32                      # band half-width
VROWS = 1152                # 32 zero pad + 1024 own + 32 halo + 64 zero tail
NT = 9                      # v tiles: 9 x 128

_CACHE = {}


def _build_nc():
    import concourse.mybir as mybir
    from concourse import bacc
    from concourse.tile import TileContext

    f32 = mybir.dt.float32
    bf16 = mybir.dt.bfloat16
    ACT_COPY = mybir.ActivationFunctionType.Copy

    nc = bacc.Bacc(None, target_bir_lowering=False)

    # host-pre-arranged layouts (SBUF partition layout; long contiguous DMA
    # descriptors instead of 256B ones):
    #   xT: [p, (c k)]  c-major, value = x^T[k*128+p, c]
    #   Wg: [p, (k inner)], Wout: [p, (h d)]
    xT = nc.dram_tensor("xT", [128, 4 * VROWS], bf16, kind="ExternalInput")
    wg = nc.dram_tensor("Wg", [128, 4 * INNER], bf16, kind="ExternalInput")
    wout = nc.dram_tensor("Wout", [128, H * D], bf16, kind="ExternalInput")
    # trimmed band pieces, [rows, (h, cols)]:
    #   A: window rows 0:128   -> chunk cols 0:128
    #   B: window rows 128:256 -> chunk cols 64:256
    #   C: window rows 256:320 -> chunk cols 192:256
    bandA = nc.dram_tensor("bandA", [128, H * 128], bf16, kind="ExternalInput")
    bandB = nc.dram_tensor("bandB", [128, H * 192], bf16, kind="ExternalInput")
    bandC = nc.dram_tensor("bandC", [64, H * 64], bf16, kind="ExternalInput")
    # edge rescale for out rows 0..31: [128(bcast), (h, 32)]
    ec = nc.dram_tensor("ec", [128, H * 32], bf16, kind="ExternalInput")
    out = nc.dram_tensor("out", [1024, D], f32, kind="ExternalOutput")

    with TileContext(nc) as tc:
        with (
            tc.tile_pool(name="persist", bufs=1) as pp,
            tc.tile_pool(name="stage", bufs=2) as sp,
            tc.tile_pool(name="psX", bufs=6, space="PSUM") as psX,
            tc.tile_pool(name="psE", bufs=2, space="PSUM") as psE,
        ):
            psW = psE  # warmup matmuls share the small ring
            xT_sb = pp.tile([128, 4 * VROWS], bf16, tag="xT", name="xT_sb")
            wg_sb = pp.tile([128, 4 * INNER], bf16, tag="wg", name="wg_sb")
            # c-major view for stage-1 lhsT slices: [p, c, k]
            xT_v = xT_sb.rearrange("p (c k) -> p c k", k=4)
            # DMA order: first-needed pieces first; everything is a plain
            # column slice of a pre-arranged [128, X] tensor (1-6KB runs).
            def dx(a, b):   # xT (c k) cols [a*512,(b)*512) = x cols [a*128,b*128)
                nc.sync.dma_start(out=xT_sb[:, a * 512:b * 512],
                                  in_=xT[:, a * 512:b * 512])

            nc.sync.dma_start(out=wg_sb[:, 0:512], in_=wg[:, 0:512])
            nc.sync.dma_start(out=wg_sb[:, 512:1024], in_=wg[:, 512:1024])
            dx(0, 1)
            nc.sync.dma_start(out=wg_sb[:, 1024:2048], in_=wg[:, 1024:2048])
            dx(1, 2)
            nc.sync.dma_start(out=wg_sb[:, 2048:3072], in_=wg[:, 2048:3072])
            nc.sync.dma_start(out=wg_sb[:, 3072:4096], in_=wg[:, 3072:4096])
            dx(2, 3)
            dx(3, 4)
            bandA_sb = pp.tile([128, H * 128], bf16, tag="bandA", name="bandA_sb")
            nc.sync.dma_start(out=bandA_sb, in_=bandA[:, :])
            dx(4, 5)
            bandC_sb = pp.tile([64, H * 64], bf16, tag="bandC", name="bandC_sb")
            nc.sync.dma_start(out=bandC_sb, in_=bandC[:, :])
            bandB_sb = pp.tile([128, H * 192], bf16, tag="bandB", name="bandB_sb")
            nc.sync.dma_start(out=bandB_sb, in_=bandB[:, :])
            dx(5, 6)
            ec_sb = pp.tile([128, H * 32], bf16, tag="ec", name="ec_sb")
            nc.sync.dma_start(out=ec_sb, in_=ec[:, :])
            dx(6, 7)
            dx(7, 9)
            wout_sb = pp.tile([128, H * D], bf16, tag="wout", name="wout_sb")
            nc.sync.dma_start(out=wout_sb, in_=wout[:, :])

            # warm the scalar-engine activation table during the prologue
            # (the first real scalar copy would otherwise eat ~1.3us of
            # ACT_TABLE_LOAD on the critical path). No DMA dependency: reads
            # the memset region.
            warm = pp.tile([128, 648], bf16, tag="warm", name="warm")
            nc.gpsimd.memset(warm[:, 8:648], 0.0)
            nc.scalar.activation(warm[:, 0:8], warm[:, 8:16], ACT_COPY)
            # PE warmup: keep the array busy from "main" start so the DVFS
            # ramp overlaps the input-DMA wait instead of stage 1.
            for w_i in range(18):
                pw = psW.tile([128, 512], f32, tag="e", name=f"warm{w_i}")
                nc.tensor.matmul(pw[:, 0:256], warm[:, 8:136],
                                 warm[:, 136:392], start=True, stop=True)

            v_sb = [pp.tile([128, INNER], bf16, tag=f"v{t}", name=f"v{t}")
                    for t in range(NT)]
            # attnT layout: [dh(128), (i(4), h(8), r(256))]
            attnT = pp.tile([128, 4 * H * 256], bf16, tag="attnT", name="attnT")

            # ---- stage-2 piece emission --------------------------------
            # chunk i (256 out rows): A = v[2i] (K=128) -> cols 0:128,
            # B = v[2i+1] -> cols 64:256, C = v[2i+2][:64] -> cols 192:256.
            # PSUM has_written clear is whole-bank on start=True: only the
            # FIRST matmul into each psum tile carries start=True.
            # Sweep 0 chunks live in the small psE ring (interleaved with
            # stage 1); sweeps 1-3 chunks in the psX ring (3 sweeps x 2
            # live chunks = 6, tile-major).
            ps_chunk = [{} for _ in range(4)]   # per sweep g: open chunks

            def copy_v(dst, src):
                nc.vector.tensor_copy(dst, src)

            def copy_s(dst, src):
                nc.scalar.activation(dst, src, ACT_COPY)

            def sweep_tile(g, t, copy_engine):
                pool, ptag = (psE, "e") if g == 0 else (psX, "x")
                hs = (2 * g, 2 * g + 1)
                for hh, h in enumerate(hs):
                    vsl = v_sb[t][:, h * 128:(h + 1) * 128]
                    if t % 2 == 0:
                        i_new, i_fin = t // 2, t // 2 - 1
                        if i_new <= 3:
                            if hh == 0:
                                ps_chunk[g][i_new] = pool.tile(
                                    [128, 512], f32, tag=ptag,
                                    name=f"ps2_{g}_{i_new}")
                            nc.tensor.matmul(
                                ps_chunk[g][i_new][:, hh * 256:hh * 256 + 128],
                                vsl, bandA_sb[:, h * 128:(h + 1) * 128],
                                start=(hh == 0), stop=False)
                        if i_fin >= 0:
                            nc.tensor.matmul(
                                ps_chunk[g][i_fin][:, hh * 256 + 192:
                                                   hh * 256 + 256],
                                v_sb[t][:64, h * 128:(h + 1) * 128],
                                bandC_sb[:, h * 64:(h + 1) * 64],
                                start=False, stop=(hh == 1))
                    else:
                        i_mid = (t - 1) // 2
                        nc.tensor.matmul(
                            ps_chunk[g][i_mid][:, hh * 256 + 64:hh * 256 + 256],
                            vsl, bandB_sb[:, h * 192:(h + 1) * 192],
                            start=False, stop=False)
                if t % 2 == 0 and t // 2 - 1 >= 0:
                    i_fin = t // 2 - 1
                    blk = attnT[:, (i_fin * 8 + 2 * g) * 256:
                                (i_fin * 8 + 2 * g + 2) * 256]
                    copy_engine(blk, ps_chunk[g].pop(i_fin))
                    if i_fin == 0:
                        # edge rescale: first 32 out rows of the sequence
                        v3 = blk.rearrange("p (h r) -> p h r", r=256)[:, :, 0:32]
                        nc.gpsimd.tensor_mul(
                            v3, v3,
                            ec_sb[:, 2 * g * 32:(2 * g + 2) * 32].rearrange(
                                "p (h r) -> p h r", r=32))

            # ---- stage 3 pair emission (chunk i -> out rows 2i*128..) ----
            def stage3_pair(i, ce0, ce1):
                for jj in range(2):
                    j = 2 * i + jj
                    half = j % 2
                    ps = psE.tile([128, 512], f32, tag="e", name=f"ps3_{j}")
                    for h in range(H):
                        off = (i * 8 + h) * 256 + half * 128
                        nc.tensor.matmul(ps, attnT[:, off:off + 128],
                                         wout_sb[:, h * D:(h + 1) * D],
                                         start=(h == 0), stop=(h == 7))
                    ot = sp.tile([128, D], f32, tag="outt", name=f"ot{j}")
                    (ce0 if jj == 0 else ce1)(ot, ps)
                    nc.sync.dma_start(out=out[j * 128:(j + 1) * 128, :], in_=ot)

            # ---- stage 1 (+ interleaved sweep 0, shifted 2 tiles so the
            # band DMAs are comfortably ahead): v tiles = xpad @ Wg
            for t in range(NT):
                psA = psX.tile([128, 512], f32, tag="x", name=f"psA{t}")
                psB = psX.tile([128, 512], f32, tag="x", name=f"psB{t}")
                for k in range(4):
                    lh = xT_v[:, t * 128:(t + 1) * 128, k]
                    nc.tensor.matmul(psA, lh, wg_sb[:, k * INNER:k * INNER + 512],
                                     start=(k == 0), stop=(k == 3))
                    nc.tensor.matmul(psB, lh,
                                     wg_sb[:, k * INNER + 512:(k + 1) * INNER],
                                     start=(k == 0), stop=(k == 3))
                nc.vector.tensor_copy(v_sb[t][:, 0:512], psA)
                nc.scalar.activation(v_sb[t][:, 512:1024], psB, ACT_COPY)
                if t >= 2:
                    sweep_tile(0, t - 2, copy_s)
            sweep_tile(0, NT - 2, copy_s)
            sweep_tile(0, NT - 1, copy_s)

            # ---- stage 2 sweeps 1..3 tile-major, stage-3 pairs inlined one
            # tile after their chunk closes (so the attnT copies have
            # settled and the tensor queue never waits on a copy)
            ci = 0
            for t in range(NT):
                for g in range(1, 4):
                    ci += 1
                    sweep_tile(g, t, copy_v if ci % 2 == 0 else copy_s)
                if t >= 3 and t % 2 == 1:
                    i = (t - 3) // 2
                    stage3_pair(i, copy_v, copy_s)
            stage3_pair(3, copy_v, copy_s)

    nc.compile()
    return nc


def _band_constants(sigma: np.ndarray):
    """Trimmed band pieces (interior row-norm baked in) + edge rescale."""
    sig = np.asarray(sigma, np.float64).reshape(H)
    d = np.arange(W + 1, dtype=np.float64)
    wts = np.exp(-(d[None, :] ** 2) / (2.0 * sig[:, None] ** 2))  # [H, 33]
    tail = wts[:, 1:].sum(1)
    s_int = wts[:, 0] + 2.0 * tail
    # edge rowsum for out rows r=0..31 (left-truncated gaussian)
    re = np.arange(32)
    cum = np.concatenate([np.zeros((H, 1)), np.cumsum(wts[:, 1:], 1)], 1)
    s_edge = wts[:, [0]] + cum[:, np.minimum(re, W)] + tail[:, None]  # [H, 32]

    bf = ml_dtypes.bfloat16

    def piece(m_count, m_off, c_lo, c_hi):
        r = np.arange(c_lo, c_hi)
        mloc = np.arange(m_count)
        dist = np.abs(r[None, :] + 32 - (m_off + mloc[:, None]))
        msk = dist <= W
        wp = np.where(msk[None], wts[:, np.minimum(dist, W).astype(int)], 0.0)
        wp = wp / s_int[:, None, None]                    # [H, m, cols]
        return np.ascontiguousarray(
            wp.transpose(1, 0, 2).reshape(m_count, H * (c_hi - c_lo))).astype(bf)

    bandA = piece(128, 0, 0, 128)
    bandB = piece(128, 128, 64, 256)
    bandC = piece(64, 256, 192, 256)
    ecv = (s_int[:, None] / s_edge).astype(bf)            # [H, 32]
    ec = np.ascontiguousarray(
        np.broadcast_to(ecv.reshape(1, H * 32), (128, H * 32))).astype(bf)
    return bandA, bandB, bandC, ec


def _in_maps(x, Wg, Wout, sigma):
    bf = ml_dtypes.bfloat16
    bandA, bandB, bandC, ec = _band_constants(sigma)
    # pre-arranged layouts: Wg [p, (k inner)], Wout [p, (h d)]
    wg = np.ascontiguousarray(
        np.asarray(Wg, np.float32).reshape(4, 128, INNER).transpose(1, 0, 2)
        .reshape(128, 4 * INNER)).astype(bf)
    wo = np.ascontiguousarray(
        np.asarray(Wout, np.float32).reshape(H, 128, D).transpose(1, 0, 2)
        .reshape(128, H * D)).astype(bf)
    x = np.asarray(x, np.float32)
    maps = []
    for c in range(8):
        b, s = divmod(c, 2)
        z = x[b] if s == 0 else x[b, ::-1]
        xbuf = np.zeros((VROWS, D), np.float32)
        xbuf[32:32 + 1056] = z[:1056]
        # xT c-major: [p, (c k)], value = x^T[k*128+p, c] = xbuf[c, k*128+p]
        xck = np.ascontiguousarray(
            xbuf.T.reshape(4, 128, VROWS).transpose(1, 2, 0)
            .reshape(128, VROWS * 4)).astype(bf)
        maps.append({
            "xT": xck,
            "Wg": wg, "Wout": wo,
            "bandA": bandA, "bandB": bandB, "bandC": bandC, "ec": ec,
        })
    return maps


def _get_nc():
    if "nc" not in _CACHE:
        _CACHE["nc"] = _build_nc()
    return _CACHE["nc"]


def run_spmd(in_maps, **kw):
    from concourse.bass_utils import run_bass_kernel_spmd
    return run_bass_kernel_spmd(_get_nc(), in_maps, core_ids=list(range(8)), **kw)


def _assemble(results):
    full = np.empty((B, N, D), np.float32)
    for c in range(8):
        b, s = divmod(c, 2)
        r = results[c]["out"]
        if s == 0:
            full[b, :1024] = r
        else:
            full[b, 1024:] = r[::-1]
    return full


def kernel(x, Wg, Wout, sigma):
    res = run_spmd(_in_maps(x, Wg, Wout, sigma))
    return _assemble(res.results)


# revision 4
# speedup vs baseline: 1.1674x; 1.1674x over previous
"""Local (Gaussian-banded) attention kernel for Trainium2, 8 NeuronCores. v2

Math: out = rownorm(gauss_band(sigma)) @ (x @ Wg) @ Wout
The Gaussian positional mask with sigma in [0.5, 2.5] decays below fp32
resolution past |i-j| > 32, so attention is a 65-tap banded matmul.

Sharding: core c = (batch b = c//2, seq-half s = c%2). Each core computes
out rows [s*1024, (s+1)*1024) of its batch. s=1 halves are row-reversed on
host so the sequence edge is always at local row 0 -> all 8 cores run the
same program with the same band constants (pure SPMD).

Performance design (measured on HW, ~59us vs 89us fp32r baseline):
  - all matmul operands bf16 (half DMA bytes, fast 95ns weight loads)
  - input DMAs interleaved at k-block granularity so stage 1 starts after
    ~0.5 MB instead of ~4.5 MB
  - stage-2 band streams trimmed to their true column support
    (A: cols 0:128, B: 64:256, C: 192:256 of each 256-row chunk)
  - sweep 0 of stage 2 interleaved into the stage-1 tile loop
  - PSUM->SBUF evacuation split across vector + scalar engines
  - scalar activation table pre-warmed during the DMA prologue
  - xT zero-padded to 1152 rows so tile 8 is a full 128-row tile
  - output DMA per 128-row chunk for early drain; the final out pair's
    matmuls are slotted into tile 8 as each sweep's chunk-3 copy lands,
    shortening the kernel tail by ~1.7us
"""

import sys

for _p in ("/opt/trn_rl_repo", "/root/.axon_site/_ro/trn_rl_repo"):
    if _p not in sys.path:
        sys.path.append(_p)

import numpy as np
import ml_dtypes

B, N, D = 4, 2048, 512
H, DH = 8, 128
INNER = H * DH
W = 32                      # band half-width
VROWS = 1152                # 32 zero pad + 1024 own + 32 halo + 64 zero tail
NT = 9                      # v tiles: 9 x 128

_CACHE = {}


def _build_nc():
    import concourse.mybir as mybir
    from concourse import bacc
    from concourse.tile import TileContext

    f32 = mybir.dt.float32
    bf16 = mybir.dt.bfloat16
    ACT_COPY = mybir.ActivationFunctionType.Copy

    nc = bacc.Bacc(None, target_bir_lowering=False)

    # host-pre-arranged layouts (SBUF partition layout; long contiguous DMA
    # descriptors instead of 256B ones):
    #   xT: [p, (c k)]  c-major, value = x^T[k*128+p, c]
    #   Wg: [p, (k inner)], Wout: [p, (h d)]
    xT = nc.dram_tensor("xT", [128, 4 * VROWS], bf16, kind="ExternalInput")
    wg = nc.dram_tensor("Wg", [128, 4 * INNER], bf16, kind="ExternalInput")
    wout = nc.dram_tensor("Wout", [128, H * D], bf16, kind="ExternalInput")
    # trimmed band pieces, [rows, (h, cols)]:
    #   A: window rows 0:128   -> chunk cols 0:128
    #   B: window rows 128:256 -> chunk cols 64:256
    #   C: window rows 256:320 -> chunk cols 192:256
    bandA = nc.dram_tensor("bandA", [128, H * 128], bf16, kind="ExternalInput")
    bandB = nc.dram_tensor("bandB", [128, H * 192], bf16, kind="ExternalInput")
    bandC = nc.dram_tensor("bandC", [64, H * 64], bf16, kind="ExternalInput")
    # edge rescale for out rows 0..31: [128(bcast), (h, 32)]
    ec = nc.dram_tensor("ec", [128, H * 32], bf16, kind="ExternalInput")
    out = nc.dram_tensor("out", [1024, D], f32, kind="ExternalOutput")

    with TileContext(nc) as tc:
        with (
            tc.tile_pool(name="persist", bufs=1) as pp,
            tc.tile_pool(name="stage", bufs=2) as sp,
            tc.tile_pool(name="psX", bufs=6, space="PSUM") as psX,
            tc.tile_pool(name="psE", bufs=2, space="PSUM") as psE,
        ):
            psW = psE  # warmup matmuls share the small ring
            xT_sb = pp.tile([128, 4 * VROWS], bf16, tag="xT", name="xT_sb")
            wg_sb = pp.tile([128, 4 * INNER], bf16, tag="wg", name="wg_sb")
            # c-major view for stage-1 lhsT slices: [p, c, k]
            xT_v = xT_sb.rearrange("p (c k) -> p c k", k=4)
            # DMA order: first-needed pieces first; everything is a plain
            # column slice of a pre-arranged [128, X] tensor (1-6KB runs).
            def dx(a, b):   # xT (c k) cols [a*512,(b)*512) = x cols [a*128,b*128)
                nc.sync.dma_start(out=xT_sb[:, a * 512:b * 512],
                                  in_=xT[:, a * 512:b * 512])

            # wg as ONE dma_start: 128 descriptors x 8KB -> byte-bound
            # (~2.6us) instead of descriptor-rate-bound; stage-1 tile 0
            # needs all of wg anyway (k accumulation).
            nc.sync.dma_start(out=wg_sb, in_=wg[:, :])
            dx(0, 1)
            dx(1, 2)
            dx(2, 3)
            dx(3, 4)
            bandA_sb = pp.tile([128, H * 128], bf16, tag="bandA", name="bandA_sb")
            nc.sync.dma_start(out=bandA_sb, in_=bandA[:, :])
            dx(4, 5)
            bandC_sb = pp.tile([64, H * 64], bf16, tag="bandC", name="bandC_sb")
            nc.sync.dma_start(out=bandC_sb, in_=bandC[:, :])
            bandB_sb = pp.tile([128, H * 192], bf16, tag="bandB", name="bandB_sb")
            nc.sync.dma_start(out=bandB_sb, in_=bandB[:, :])
            dx(5, 6)
            ec_sb = pp.tile([128, H * 32], bf16, tag="ec", name="ec_sb")
            nc.sync.dma_start(out=ec_sb, in_=ec[:, :])
            dx(6, 7)
            dx(7, 9)
            wout_sb = pp.tile([128, H * D], bf16, tag="wout", name="wout_sb")
            nc.sync.dma_start(out=wout_sb, in_=wout[:, :])

            # warm the scalar-engine activation table during the prologue
            # (the first real scalar copy would otherwise eat ~1.3us of
            # ACT_TABLE_LOAD on the critical path). No DMA dependency: reads
            # the memset region.
            warm = pp.tile([128, 648], bf16, tag="warm", name="warm")
            nc.gpsimd.memset(warm[:, 8:648], 0.0)
            nc.scalar.activation(warm[:, 0:8], warm[:, 8:16], ACT_COPY)
            # PE warmup: keep the array busy from "main" start so the DVFS
            # ramp overlaps the input-DMA wait instead of stage 1.
            for w_i in range(18):
                pw = psW.tile([128, 512], f32, tag="e", name=f"warm{w_i}")
                nc.tensor.matmul(pw[:, 0:256], warm[:, 8:136],
                                 warm[:, 136:392], start=True, stop=True)

            v_sb = [pp.tile([128, INNER], bf16, tag=f"v{t}", name=f"v{t}")
                    for t in range(NT)]
            # attnT layout: [dh(128), (i(4), h(8), r(256))]
            attnT = pp.tile([128, 4 * H * 256], bf16, tag="attnT", name="attnT")

            # ---- stage-2 piece emission --------------------------------
            # chunk i (256 out rows): A = v[2i] (K=128) -> cols 0:128,
            # B = v[2i+1] -> cols 64:256, C = v[2i+2][:64] -> cols 192:256.
            # PSUM has_written clear is whole-bank on start=True: only the
            # FIRST matmul into each psum tile carries start=True.
            # Sweep 0 chunks live in the small psE ring (interleaved with
            # stage 1); sweeps 1-3 chunks in the psX ring (3 sweeps x 2
            # live chunks = 6, tile-major).
            ps_chunk = [{} for _ in range(4)]   # per sweep g: open chunks

            def copy_v(dst, src):
                nc.vector.tensor_copy(dst, src)

            def copy_s(dst, src):
                nc.scalar.activation(dst, src, ACT_COPY)

            def sweep_tile(g, t, copy_engine):
                pool, ptag = (psE, "e") if g == 0 else (psX, "x")
                hs = (2 * g, 2 * g + 1)
                for hh, h in enumerate(hs):
                    vsl = v_sb[t][:, h * 128:(h + 1) * 128]
                    if t % 2 == 0:
                        i_new, i_fin = t // 2, t // 2 - 1
                        if i_new <= 3:
                            if hh == 0:
                                ps_chunk[g][i_new] = pool.tile(
                                    [128, 512], f32, tag=ptag,
                                    name=f"ps2_{g}_{i_new}")
                            nc.tensor.matmul(
                                ps_chunk[g][i_new][:, hh * 256:hh * 256 + 128],
                                vsl, bandA_sb[:, h * 128:(h + 1) * 128],
                                start=(hh == 0), stop=False)
                        if i_fin >= 0:
                            nc.tensor.matmul(
                                ps_chunk[g][i_fin][:, hh * 256 + 192:
                                                   hh * 256 + 256],
                                v_sb[t][:64, h * 128:(h + 1) * 128],
                                bandC_sb[:, h * 64:(h + 1) * 64],
                                start=False, stop=(hh == 1))
                    else:
                        i_mid = (t - 1) // 2
                        nc.tensor.matmul(
                            ps_chunk[g][i_mid][:, hh * 256 + 64:hh * 256 + 256],
                            vsl, bandB_sb[:, h * 192:(h + 1) * 192],
                            start=False, stop=False)
                if t % 2 == 0 and t // 2 - 1 >= 0:
                    i_fin = t // 2 - 1
                    blk = attnT[:, (i_fin * 8 + 2 * g) * 256:
                                (i_fin * 8 + 2 * g + 2) * 256]
                    copy_engine(blk, ps_chunk[g].pop(i_fin))
                    if i_fin == 0:
                        # edge rescale: first 32 out rows of the sequence
                        v3 = blk.rearrange("p (h r) -> p h r", r=256)[:, :, 0:32]
                        nc.gpsimd.tensor_mul(
                            v3, v3,
                            ec_sb[:, 2 * g * 32:(2 * g + 2) * 32].rearrange(
                                "p (h r) -> p h r", r=32))

            # ---- stage 3 pair emission (chunk i -> out rows 2i*128..) ----
            def stage3_pair(i, ce0, ce1):
                for jj in range(2):
                    j = 2 * i + jj
                    half = j % 2
                    ps = psE.tile([128, 512], f32, tag="e", name=f"ps3_{j}")
                    for h in range(H):
                        off = (i * 8 + h) * 256 + half * 128
                        nc.tensor.matmul(ps, attnT[:, off:off + 128],
                                         wout_sb[:, h * D:(h + 1) * D],
                                         start=(h == 0), stop=(h == 7))
                    ot = sp.tile([128, D], f32, tag="outt", name=f"ot{j}")
                    (ce0 if jj == 0 else ce1)(ot, ps)
                    nc.sync.dma_start(out=out[j * 128:(j + 1) * 128, :], in_=ot)

            # ---- stage 1 (+ interleaved sweep 0, shifted 2 tiles so the
            # band DMAs are comfortably ahead): v tiles = xpad @ Wg
            for t in range(NT):
                psA = psX.tile([128, 512], f32, tag="x", name=f"psA{t}")
                psB = psX.tile([128, 512], f32, tag="x", name=f"psB{t}")
                for k in range(4):
                    lh = xT_v[:, t * 128:(t + 1) * 128, k]
                    nc.tensor.matmul(psA, lh, wg_sb[:, k * INNER:k * INNER + 512],
                                     start=(k == 0), stop=(k == 3))
                    nc.tensor.matmul(psB, lh,
                                     wg_sb[:, k * INNER + 512:(k + 1) * INNER],
                                     start=(k == 0), stop=(k == 3))
                nc.vector.tensor_copy(v_sb[t][:, 0:512], psA)
                nc.scalar.activation(v_sb[t][:, 512:1024], psB, ACT_COPY)
                if t >= 2:
                    sweep_tile(0, t - 2, copy_s)
            sweep_tile(0, NT - 2, copy_s)
            sweep_tile(0, NT - 1, copy_s)

            # ---- stage 2 sweeps 1..3 tile-major, stage-3 pairs inlined one
            # tile after their chunk closes (so the attnT copies have
            # settled and the tensor queue never waits on a copy)
            ci = 0
            ps_j67 = {}

            def pair3_heads(hs):
                # final pair (out rows 768:1024) interleaved into tile 8:
                # each head-pair's matmuls run as soon as its sweep's chunk-3
                # attnT copy lands, shortening the kernel tail.
                for jj in range(2):
                    j = 6 + jj
                    if hs[0] == 0:
                        ps_j67[jj] = psE.tile([128, 512], f32, tag="e",
                                              name=f"ps3_{j}")
                    for h in hs:
                        off = (3 * 8 + h) * 256 + (j % 2) * 128
                        nc.tensor.matmul(ps_j67[jj], attnT[:, off:off + 128],
                                         wout_sb[:, h * D:(h + 1) * D],
                                         start=(h == 0), stop=(h == 7))

            for t in range(NT):
                for g in range(1, 4):
                    ci += 1
                    sweep_tile(g, t, copy_v if ci % 2 == 0 else copy_s)
                    if t == NT - 1 and g >= 2:
                        # chunk-3 attnT copies land progressively at tile 8;
                        # slot the final pair's head-matmuls in as their
                        # sweep's copy fires (sweep 0/1 slices first) so the
                        # tail never waits on a copy
                        if g == 2:
                            pair3_heads((0, 1))
                            pair3_heads((2, 3))
                        else:
                            pair3_heads((4, 5))
                            pair3_heads((6, 7))
                if t >= 3 and t % 2 == 1:
                    i = (t - 3) // 2
                    stage3_pair(i, copy_v, copy_s)
            for jj in range(2):
                j = 6 + jj
                ot = sp.tile([128, D], f32, tag="outt", name=f"ot{j}")
                (copy_v if jj == 0 else copy_s)(ot, ps_j67[jj])
                nc.sync.dma_start(out=out[j * 128:(j + 1) * 128, :], in_=ot)

    nc.compile()
    return nc


def _band_constants(sigma: np.ndarray):
    """Trimmed band pieces (interior row-norm baked in) + edge rescale."""
    sig = np.asarray(sigma, np.float64).reshape(H)
    d = np.arange(W + 1, dtype=np.float64)
    wts = np.exp(-(d[None, :] ** 2) / (2.0 * sig[:, None] ** 2))  # [H, 33]
    tail = wts[:, 1:].sum(1)
    s_int = wts[:, 0] + 2.0 * tail
    # edge rowsum for out rows r=0..31 (left-truncated gaussian)
    re = np.arange(32)
    cum = np.concatenate([np.zeros((H, 1)), np.cumsum(wts[:, 1:], 1)], 1)
    s_edge = wts[:, [0]] + cum[:, np.minimum(re, W)] + tail[:, None]  # [H, 32]

    bf = ml_dtypes.bfloat16

    def piece(m_count, m_off, c_lo, c_hi):
        r = np.arange(c_lo, c_hi)
        mloc = np.arange(m_count)
        dist = np.abs(r[None, :] + 32 - (m_off + mloc[:, None]))
        msk = dist <= W
        wp = np.where(msk[None], wts[:, np.minimum(dist, W).astype(int)], 0.0)
        wp = wp / s_int[:, None, None]                    # [H, m, cols]
        return np.ascontiguousarray(
            wp.transpose(1, 0, 2).reshape(m_count, H * (c_hi - c_lo))).astype(bf)

    bandA = piece(128, 0, 0, 128)
    bandB = piece(128, 128, 64, 256)
    bandC = piece(64, 256, 192, 256)
    ecv = (s_int[:, None] / s_edge).astype(bf)            # [H, 32]
    ec = np.ascontiguousarray(
        np.broadcast_to(ecv.reshape(1, H * 32), (128, H * 32))).astype(bf)
    return bandA, bandB, bandC, ec


def _in_maps(x, Wg, Wout, sigma):
    bf = ml_dtypes.bfloat16
    bandA, bandB, bandC, ec = _band_constants(sigma)
    # pre-arranged layouts: Wg [p, (k inner)], Wout [p, (h d)]
    wg = np.ascontiguousarray(
        np.asarray(Wg, np.float32).reshape(4, 128, INNER).transpose(1, 0, 2)
        .reshape(128, 4 * INNER)).astype(bf)
    wo = np.ascontiguousarray(
        np.asarray(Wout, np.float32).reshape(H, 128, D).transpose(1, 0, 2)
        .reshape(128, H * D)).astype(bf)
    x = np.asarray(x, np.float32)
    maps = []
    for c in range(8):
        b, s = divmod(c, 2)
        z = x[b] if s == 0 else x[b, ::-1]
        xbuf = np.zeros((VROWS, D), np.float32)
        xbuf[32:32 + 1056] = z[:1056]
        # xT c-major: [p, (c k)], value = x^T[k*128+p, c] = xbuf[c, k*128+p]
        xck = np.ascontiguousarray(
            xbuf.T.reshape(4, 128, VROWS).transpose(1, 2, 0)
            .reshape(128, VROWS * 4)).astype(bf)
        maps.append({
            "xT": xck,
            "Wg": wg, "Wout": wo,
            "bandA": bandA, "bandB": bandB, "bandC": bandC, "ec": ec,
        })
    return maps


def _get_nc():
    if "nc" not in _CACHE:
        _CACHE["nc"] = _build_nc()
    return _CACHE["nc"]


def run_spmd(in_maps, **kw):
    from concourse.bass_utils import run_bass_kernel_spmd
    return run_bass_kernel_spmd(_get_nc(), in_maps, core_ids=list(range(8)), **kw)


def _assemble(results):
    full = np.empty((B, N, D), np.float32)
    for c in range(8):
        b, s = divmod(c, 2)
        r = results[c]["out"]
        if s == 0:
            full[b, :1024] = r
        else:
            full[b, 1024:] = r[::-1]
    return full


def kernel(x, Wg, Wout, sigma):
    res = run_spmd(_in_maps(x, Wg, Wout, sigma))
    return _assemble(res.results)
